# revision 24
# baseline (speedup 1.0000x reference)
"""Trainium2 Bass kernel for nn_AsymmetricProjectedLinear (8 NeuronCores).

Reference computes out = x @ W_large^T with
    W_large = (A_out @ B_out) @ W_small @ (A_in @ B_in)^T    [4096, 4096]

W_large (137 GFLOP naive) is never materialized. Factored (~4.5 GFLOP):
    M   = B_in @ W_small^T @ B_out^T            [64, 64]   (tiny)
    out = ((x @ A_in) @ M) @ A_out^T            [4096t, 4096]

Sharding: tokens (B*S = 4096) split 512/core across 8 cores; weights
replicated. Host work is layout-only (transpose/pack/slice/dtype-cast);
all FLOPs run on the NeuronCores.

v4 design (driven by v1-v3 perfetto traces; the kernel is jointly
wire-bound (~11.7MB over two ~215GB/s HWDGE rings) and PE-bound when
the PE runs at its cold 1.2GHz clock):
  - Stream order (each tensor split half/half over both rings, FIFO):
      wsm | W | x(B0) | a_outT | x(B1) | out(B0) | out(B1)
    W rides ahead of x so the M prework finishes while x(B0) streams.
  - Stage 1 is column-packed: even k-tiles accumulate in PE column
    strip 0 (PSUM partitions 0-63), odd k-tiles in strip 1 (64-127),
    doubling effective matmul rate. The half-sums are combined for
    free in stage 2 by stacking M twice along K (m_q = [[M,M],[M,M]]),
    which also lands t2 duplicated into both PSUM partition halves for
    stage 5's row-packing - so no cross-partition copies anywhere.
  - A short burst of junk matmuls on memset data warms the PE HAM
    clock gate (1.2 -> 2.4GHz) before real work arrives, and the
    k-chasing schedule keeps the PE dense so it stays warm.
  - Stage 5 row-packs the two 128-token halves in the two PE row
    halves; PSUM drained by DVE (lo) + ACT (hi); outputs stream in
    0.25MB chunks as soon as each pair of o-slices is drained.
  - bf16 everywhere (rel err ~5e-3 vs the 2e-2 harness gate); PSUM
    accumulation stays fp32.
"""

import numpy as np

import concourse.bass as bass
import concourse.mybir as mybir
import concourse.tile as tile
from concourse import bacc
from concourse.bass_utils import run_bass_kernel_spmd

N_CORES = 8
Bsz, S, D = 2, 2048, 4096
TOK = Bsz * S          # 4096 tokens
T = TOK // N_CORES     # 512 tokens per core
TB = 256               # tokens per stage-1 block
NBLK = T // TB         # 2 blocks
RANK = 64
DS = 1024              # d_small
KT = D // 128          # 32 k-tiles over d_in_large

F32 = mybir.dt.float32
BF16 = mybir.dt.bfloat16
OUT_DT = BF16

# wsm packed columns: b_outT | b_inT | a_in | ident
C_BOT = 0
C_BIT = C_BOT + 8 * RANK
C_AIN = C_BIT + 8 * RANK
C_IDT = C_AIN + KT * RANK
C_WSM = C_IDT + RANK
C_SPLIT = C_AIN + 9 * RANK   # ring split boundary (1600 / 1536 cols)

_nc_cache = {}


def build():
    if "nc" in _nc_cache:
        return _nc_cache["nc"]
    nc = bacc.Bacc("TRN2", target_bir_lowering=False, debug=False,
                   num_devices=N_CORES)

    # x_p: [NBLK, 4 pieces, 2 rings, 128, 4 k-tiles * TB]
    x_p = nc.dram_tensor("x_p", [NBLK, 4, 2, 128, 4 * TB], BF16,
                         kind="ExternalInput")
    wsm_p = nc.dram_tensor("wsm_p", [128, C_WSM], BF16, kind="ExternalInput")
    w_p = nc.dram_tensor("w_p", [128, 8 * DS], BF16, kind="ExternalInput")
    # a_outT_p: [128, D] with rows 0-63 and 64-127 both = A_out^T
    a_outT_p = nc.dram_tensor("a_outT_p", [128, D], BF16, kind="ExternalInput")
    out = nc.dram_tensor("out", [T, D], OUT_DT, kind="ExternalOutput")

    with tile.TileContext(nc) as tc:
        with (
            tc.tile_pool(name="const", bufs=1) as cpool,
            tc.tile_pool(name="xin", bufs=2) as xpool,
            tc.tile_pool(name="outp", bufs=2) as opool,
            tc.tile_pool(name="interm", bufs=2) as ipool,
            tc.tile_pool(name="ps_u", bufs=1, space="PSUM") as ps_u,
            tc.tile_pool(name="ps_g", bufs=1, space="PSUM") as ps_g,
            tc.tile_pool(name="ps_gt", bufs=1, space="PSUM") as ps_gt,
            tc.tile_pool(name="ps_t2", bufs=1, space="PSUM") as ps_t2,
            tc.tile_pool(name="ps_mp", bufs=1, space="PSUM") as ps_mp,
            tc.tile_pool(name="ps_o", bufs=3, space="PSUM") as ps_o,
        ):
            rings = (nc.sync, nc.scalar)

            # ---- constant tiles --------------------------------------
            wsm_s = cpool.tile([128, C_WSM], BF16)
            b_outT_s = wsm_s[:, C_BOT:C_BOT + 8 * RANK]
            b_inT_s = wsm_s[:, C_BIT:C_BIT + 8 * RANK]
            a_in_s = wsm_s[:, C_AIN:C_AIN + KT * RANK]
            # identity replicated in both partition halves (host side)
            ident_h = (wsm_s[:RANK, C_IDT:C_IDT + RANK],
                       wsm_s[RANK:128, C_IDT:C_IDT + RANK])
            w_tile = cpool.tile([128, 8 * DS], BF16)
            a_outT_s = cpool.tile([128, D], BF16)
            jnk_s = cpool.tile([128, 576], BF16)
            x_tiles = [[None] * 8 for _ in range(NBLK)]

            # ---- ring emission (FIFO order = stream order) -----------
            def dma_wsm():
                rings[0].dma_start(out=wsm_s[:, :C_SPLIT],
                                   in_=wsm_p.ap()[:, :C_SPLIT])
                rings[1].dma_start(out=wsm_s[:, C_SPLIT:],
                                   in_=wsm_p.ap()[:, C_SPLIT:])

            def dma_w():
                # 0.5MB pieces so G's j-chase starts at the first piece
                # (a DMA's completion sem covers the whole transfer).
                for h in range(2):
                    for r in range(2):
                        sl = slice((r * 2 + h) * 2 * DS,
                                   (r * 2 + h + 1) * 2 * DS)
                        rings[r].dma_start(out=w_tile[:, sl],
                                           in_=w_p.ap()[:, sl])

            def dma_x(b):
                for p in range(4):
                    for r in range(2):
                        xt = xpool.tile([128, 4 * TB], BF16,
                                        tag=f"x{p * 2 + r}")
                        rings[r].dma_start(out=xt[:, :],
                                           in_=x_p.ap()[b, p, r, :, :])
                        x_tiles[b][p * 2 + r] = xt

            def dma_a_outT():
                # quarters: stage 5 o-slices chase piece arrival
                for h in range(2):
                    for r in range(2):
                        sl = slice((r * 2 + h) * 1024, (r * 2 + h + 1) * 1024)
                        rings[r].dma_start(out=a_outT_s[:, sl],
                                           in_=a_outT_p.ap()[:, sl])

            def dma_out_chunk(b, o_t0, o_t1, c):
                # one 0.25MB chunk per tile (o-slices 2c, 2c+1), rings
                # alternating; emitted right after those slices drain so
                # neither ring engine head-blocks on later copies.
                r0 = b * TB
                sl = slice(c * 1024, (c + 1) * 1024)
                rings[c % 2].dma_start(
                    out=out.ap()[r0:r0 + 128, sl], in_=o_t0[:, sl])
                rings[(c + 1) % 2].dma_start(
                    out=out.ap()[r0 + 128:r0 + TB, sl], in_=o_t1[:, sl])

            # ---- compute ---------------------------------------------
            def warmup():
                # PE HAM needs ~3.4us of sustained busy to lift the
                # clock gate to 8/8; burn junk matmuls on memset data
                # while the first DMAs stream. Results are discarded.
                nc.gpsimd.memset(jnk_s[:, :], 0.0)
                for i in range(9):
                    jp = ps_t2.tile([RANK, 512], F32, tag="t2ps")
                    nc.tensor.matmul(jp[:, :], jnk_s[:, 0:RANK],
                                     jnk_s[:, 64:576], start=True, stop=True)

            def prework_g():
                # G = B_out @ W_small [64, DS], column-packed over the
                # two 512-col halves: h=0 lands on PSUM partitions
                # 0-63, h=1 on 64-127 (concurrent PE column strips,
                # one bank). j interleaved to match the two rings'
                # arrival order (sync j0-3, scalar j4-7).
                g_ps = ps_g.tile([128, 512], F32, tag="ps_g")
                jorder = [0, 4, 1, 5, 2, 6, 3, 7]
                for idx, j in enumerate(jorder):
                    for h in range(2):
                        nc.tensor.matmul(
                            g_ps[h * RANK:(h + 1) * RANK, :],
                            b_outT_s[:, j * RANK:(j + 1) * RANK],
                            w_tile[:, j * DS + h * 512:j * DS + (h + 1) * 512],
                            start=(idx == 0), stop=(idx == 7),
                        )
                g_s = ipool.tile([128, 512], BF16, tag="g")
                nc.vector.tensor_copy(g_s[:, :], g_ps[:, :])
                return g_s

            def prework_m(g_s):
                # Move G's h=1 half (PSUM partitions 64-127) down to
                # 0-63 with one identity matmul (PE transposes only
                # support base-0 inputs), then G^T via PE transpose (4
                # chunks per half into one PSUM tile, one drain copy),
                # then M = B_in @ G^T, computed twice via PE column
                # strips so M lands on partitions 0-63 AND 64-127.
                g2_ps = ps_g.tile([RANK, 512], F32, tag="ps_g")
                nc.tensor.matmul(g2_ps[:, :], ident_h[1][:, :],
                                 g_s[RANK:128, :], start=True, stop=True)
                g2_s = ipool.tile([RANK, 512], BF16, tag="g2")
                nc.vector.tensor_copy(g2_s[:, :], g2_ps[:, :])
                gT_s = ipool.tile([128, 8 * RANK], BF16, tag="gT")
                gt_ps = ps_gt.tile([128, 8 * RANK], BF16, tag="gt")
                for it in range(8):
                    h, c = it // 4, it % 4
                    src = g_s if h == 0 else g2_s
                    nc.tensor.transpose(
                        gt_ps[:, it * RANK:(it + 1) * RANK],
                        src[0:RANK, c * 128:(c + 1) * 128],
                        ident_h[0][:, :])
                nc.vector.tensor_copy(gT_s[:, :], gt_ps[:, :])
                m_ps = ps_mp.tile([128, RANK], F32, tag="mps")
                for it in range(8):
                    # d-chunk of gT tile it: covers d_small columns
                    # (it//4)*512 + (it%4)*128 .. +128 of G, i.e.
                    # b_inT tile index (it//4)*4 + (it%4) = it
                    for half in range(2):
                        nc.tensor.matmul(
                            m_ps[half * RANK:(half + 1) * RANK, :],
                            b_inT_s[:, it * RANK:(it + 1) * RANK],
                            gT_s[:, it * RANK:(it + 1) * RANK],
                            start=(it == 0), stop=(it == 7),
                        )
                # m_q [128, 128] = [[M, M], [M, M]]: K=128 folds the
                # two stage-1 column-strip partial sums; duplicated
                # cols land t2 in both PSUM halves.
                m_q = ipool.tile([128, 128], BF16, tag="mq")
                nc.vector.tensor_copy(m_q[:, 0:RANK], m_ps[:, :])
                nc.vector.tensor_copy(m_q[:, RANK:128], m_ps[:, :])
                return m_q

            def stage1(b):
                # column-packed: even k-tiles -> PSUM partitions 0-63,
                # odd k-tiles -> partitions 64-127 (concurrent strips).
                u1 = ps_u.tile([128, TB], F32, tag="u1")
                for i in range(KT // 2):
                    me, mo = 2 * i, 2 * i + 1
                    xt = x_tiles[b][me // 4]
                    ke, ko = me % 4, mo % 4
                    nc.tensor.matmul(
                        u1[0:RANK, :],
                        a_in_s[:, me * RANK:(me + 1) * RANK],
                        xt[:, ke * TB:(ke + 1) * TB],
                        start=(i == 0), stop=(i == KT // 2 - 1),
                    )
                    nc.tensor.matmul(
                        u1[RANK:128, :],
                        a_in_s[:, mo * RANK:(mo + 1) * RANK],
                        xt[:, ko * TB:(ko + 1) * TB],
                        start=(i == 0), stop=(i == KT // 2 - 1),
                    )
                u1_s = ipool.tile([128, TB], BF16, tag="u1s")
                nc.vector.tensor_copy(u1_s[:, :], u1[:, :])
                return u1_s

            def stage2(u1_s, m_q):
                t2_ps = ps_t2.tile([128, TB], F32, tag="t2ps")
                nc.tensor.matmul(t2_ps[:, :], m_q[:, :], u1_s[:, :],
                                 start=True, stop=True)
                t2_s = ipool.tile([128, TB], BF16, tag="t2s")
                nc.vector.tensor_copy(t2_s[:, :], t2_ps[:, :])
                return t2_s

            def stage5_slice(t2_s, o_t0, o_t1, o):
                sl = slice(o * 512, (o + 1) * 512)
                po0 = ps_o.tile([128, 512], F32, tag="ps_out")
                po1 = ps_o.tile([128, 512], F32, tag="ps_out")
                nc.tensor.matmul(
                    po0[:, :], t2_s[0:RANK, 0:128],
                    a_outT_s[0:RANK, sl], start=True, stop=True)
                nc.tensor.matmul(
                    po1[:, :], t2_s[RANK:128, 128:TB],
                    a_outT_s[RANK:128, sl], start=True, stop=True)
                nc.vector.tensor_copy(o_t0[:, sl], po0[:, :])
                nc.scalar.copy(o_t1[:, sl], po1[:, :])

            # ---- emission --------------------------------------------
            dma_wsm()
            dma_w()
            dma_x(0)
            dma_a_outT()
            dma_x(1)

            warmup()
            g_s = prework_g()
            u1_b0 = stage1(0)
            m_q = prework_m(g_s)
            t2_b0 = stage2(u1_b0, m_q)
            # interleave stage5(B0) with stage1(B1) so the PE chases
            # the x(B1) stream while draining B0's outputs.
            o00 = opool.tile([128, D], OUT_DT, tag="ot0")
            o01 = opool.tile([128, D], OUT_DT, tag="ot1")
            u1_ps_b1 = ps_u.tile([128, TB], F32, tag="u1")
            for o in range(8):
                stage5_slice(t2_b0, o00, o01, o)
                if o % 2 == 1:
                    dma_out_chunk(0, o00, o01, o // 2)
                for i in (2 * o, 2 * o + 1):
                    me, mo = 2 * i, 2 * i + 1
                    xt = x_tiles[1][me // 4]
                    ke, ko = me % 4, mo % 4
                    nc.tensor.matmul(
                        u1_ps_b1[0:RANK, :],
                        a_in_s[:, me * RANK:(me + 1) * RANK],
                        xt[:, ke * TB:(ke + 1) * TB],
                        start=(i == 0), stop=(i == KT // 2 - 1),
                    )
                    nc.tensor.matmul(
                        u1_ps_b1[RANK:128, :],
                        a_in_s[:, mo * RANK:(mo + 1) * RANK],
                        xt[:, ko * TB:(ko + 1) * TB],
                        start=(i == 0), stop=(i == KT // 2 - 1),
                    )
            u1_b1 = ipool.tile([128, TB], BF16, tag="u1s")
            nc.vector.tensor_copy(u1_b1[:, :], u1_ps_b1[:, :])
            t2_b1 = stage2(u1_b1, m_q)
            o10 = opool.tile([128, D], OUT_DT, tag="ot0")
            o11 = opool.tile([128, D], OUT_DT, tag="ot1")
            for o in range(8):
                stage5_slice(t2_b1, o10, o11, o)
                if o % 2 == 1:
                    dma_out_chunk(1, o10, o11, o // 2)

    nc.compile()
    _nc_cache["nc"] = nc
    return nc


def _prep_in_maps(x, W_small, A_out, B_out, A_in, B_in):
    import ml_dtypes
    f = ml_dtypes.bfloat16
    x2 = np.asarray(x, dtype=f).reshape(TOK, D)
    a_in_p = np.ascontiguousarray(
        np.asarray(A_in, f).reshape(KT, 128, RANK).transpose(1, 0, 2)
    ).reshape(128, KT * RANK)
    b_inT_p = np.ascontiguousarray(
        np.asarray(B_in, f).T.reshape(8, 128, RANK).transpose(1, 0, 2)
    ).reshape(128, 8 * RANK)
    b_outT_p = np.ascontiguousarray(
        np.asarray(B_out, f).T.reshape(8, 128, RANK).transpose(1, 0, 2)
    ).reshape(128, 8 * RANK)
    ident = np.zeros((128, RANK), f)
    ident[:RANK] = np.eye(RANK, dtype=f)
    ident[RANK:] = np.eye(RANK, dtype=f)
    wsm_p = np.ascontiguousarray(
        np.concatenate([b_outT_p, b_inT_p, a_in_p, ident], axis=1))
    w_p = np.ascontiguousarray(
        np.asarray(W_small, f).reshape(8, 128, DS).transpose(1, 0, 2)
    ).reshape(128, 8 * DS)
    aoT = np.asarray(A_out, f).T                     # [64, D]
    a_outT_p = np.ascontiguousarray(np.concatenate([aoT, aoT], axis=0))

    shared = {"wsm_p": wsm_p, "w_p": w_p, "a_outT_p": a_outT_p}
    in_maps = []
    for c in range(N_CORES):
        xs = x2[c * T:(c + 1) * T, :]                # [T, D]
        # chunk (B, p, r): tokens [B*TB,(B+1)*TB), k-tiles p*8+r*4 ..+4
        xp = np.ascontiguousarray(
            xs.T                                     # [D, T]
            .reshape(4, 2, 4, 128, NBLK, TB)         # p, r, kk, part, B, t
            .transpose(4, 0, 1, 3, 2, 5)             # B, p, r, part, kk, t
        ).reshape(NBLK, 4, 2, 128, 4 * TB)
        in_maps.append({"x_p": xp, **shared})
    return in_maps


def _run(inputs, trace=False):
    nc = build()
    in_maps = _prep_in_maps(**inputs)
    res = run_bass_kernel_spmd(
        nc, in_maps, core_ids=list(range(N_CORES)), trace=trace
    )
    out = np.concatenate(
        [np.asarray(res.results[c]["out"], dtype=np.float32)
         for c in range(N_CORES)], axis=0
    ).reshape(Bsz, S, D)
    return out, res


def kernel(**inputs) -> np.ndarray:
    out, _ = _run(inputs, trace=False)
    return out
